# revision 38
# baseline (speedup 1.0000x reference)
"""Multi-head attention (B=2, N=2048, C=768, H=12) on 8 TRN2 NeuronCores.

Sharding: batch x head-group. core = b*4 + g handles batch b, heads
(a, b, c) = (3g, 3g+1, 3g+2). Each core computes qkv projection for its
heads, attention, and a partial output projection (row-sharded W_proj);
host sums the 4 partials per batch.

Device algorithm (per core), all matmuls fp32r:
  stage A (streamed by n-quarters of 512, x DMA'd per quarter):
    qk proj into [d, n] layout: 3 groups [q_a|q_b], [k_a|k_b], [q_c|k_c]
    (host-permuted W columns so each head's q and k share a partition
    base: a -> 0, b -> 64; k_c realigned to base 0 via one SB2SB DMA)
    v proj directly into [n, d] layout (stationary = x chunk, moving =
    W cols 320:576 = [k_c pad | v], 256 wide for full fp32r rate), with
    a ones column per nk chunk appended (softmax denominator trick)
    head-a / block-0 attention interleaved into stage A so the ACT
    engine (exp) starts early
  steady state per (head h, nq-block nb of 512):
    scores S^T chunks = kT' qT (PSUM, 2x512 per piece), exp via ACT
    (scale=1/8, no max subtraction: logits ~ N(0,1))
    prior block's projection emitted between scores and AV (fills PE
    while ACT works), then AV: out_aug = [v|1]^T P^T over 16 nk chunks
    normalize: reciprocal (DVE) -> partition_broadcast from partition 64
    (Pool) -> multiply (DVE); head c shifted to outT12[64:128] via DMA
  projection: per 128-row chunk, 2 pieces of 384 cols; ff (PSUM) ->
    fs (SBUF, Pool copy) -> DRAM
"""
import sys
sys.path.insert(0, "/opt/trn_rl_repo")
import numpy as np

HEADS = 12
C = 768
D = 64
N = 2048
B = 2
NH = 3            # heads per core
SCALE = D ** -0.5
NCORES = 8

_cache = {}


def _build():
    import concourse.bass as bass
    import concourse.mybir as mybir
    from concourse import bacc
    from concourse.tile import TileContext
    from contextlib import ExitStack

    FP32 = mybir.dt.float32
    FP32R = mybir.dt.float32r
    BF16 = mybir.dt.bfloat16
    AF = mybir.ActivationFunctionType

    nc = bacc.Bacc("TRN2", target_bir_lowering=False, debug=False,
                   num_devices=NCORES)
    x_d = nc.declare_dram_parameter("x", [C, N], FP32R, isOutput=False)
    wqkvT_d = nc.declare_dram_parameter("wqkvT", [C, 576], FP32R,
                                        isOutput=False)
    wpT_d = nc.declare_dram_parameter("wpT", [192, C], FP32R, isOutput=False)
    out_d = nc.declare_dram_parameter("out", [N, C], FP32, isOutput=True)

    QB = 512          # nq block size (== n quarter)
    NKC = 16          # nk chunks of 128

    with TileContext(nc) as tc, ExitStack() as ctx:
        persist = ctx.enter_context(tc.tile_pool(name="persist", bufs=1))
        ones = persist.tile([128, 64], FP32, name="ones")
        wq_sb = persist.tile([128, 6, 576], FP32R, name="wq_sb")
        # q/k group tiles per quarter: G0=[q_a|q_b] G1=[k_a|k_b] G2=[q_c|k_c]
        G = [[persist.tile([128, QB], FP32R, name=f"G{g}q{q}")
              for q in range(4)] for g in range(3)]
        algk2 = [persist.tile([64, QB], FP32R, name=f"algk2q{q}")
                 for q in range(4)]
        # v in nk-partition layout, 65th col = ones, all heads in one tile
        v_aug = persist.tile([128, NH * NKC * 65], BF16, name="v_aug")
        vv = v_aug.rearrange("p (h c w) -> p h c w", h=NH, w=65)
        outT0 = persist.tile([64, N], FP32R, name="outT0")
        outT12 = persist.tile([128, N], FP32R, name="outT12")
        wp0 = persist.tile([64, C], FP32R, name="wp0")
        wp12 = persist.tile([128, C], FP32R, name="wp12")

        xq_pool = ctx.enter_context(tc.tile_pool(name="xq", bufs=2))
        pt_pool = ctx.enter_context(tc.tile_pool(name="pt", bufs=24))
        rs_pool = ctx.enter_context(tc.tile_pool(name="rs", bufs=2))
        bc_pool = ctx.enter_context(tc.tile_pool(name="bc", bufs=2))
        fs_pool = ctx.enter_context(tc.tile_pool(name="fs", bufs=4))
        o2s_pool = ctx.enter_context(tc.tile_pool(name="o2s", bufs=2))
        ps_s = ctx.enter_context(tc.tile_pool(name="ps_s", bufs=2,
                                              space="PSUM"))
        ps_o = ctx.enter_context(tc.tile_pool(name="ps_o", bufs=2,
                                              space="PSUM"))

        def kT(h, c):          # stationary [64, 128], nk chunk c
            q, loc = divmod(c, 4)
            csl = slice(loc * 128, (loc + 1) * 128)
            if h == 0:
                return G[1][q][0:64, csl]
            if h == 1:
                return G[1][q][64:128, csl]
            return algk2[q][:, csl]

        def qT(h, nb):         # moving [64, 512], nq block nb
            if h == 0:
                return G[0][nb][0:64, :]
            if h == 1:
                return G[0][nb][64:128, :]
            return G[2][nb][0:64, :]

        def scores_piece(h, nb, j):
            ss = ps_s.tile([128, 1024], FP32, name="ss", tag="ss")
            for t in (0, 1):
                c = 2 * j + t
                nc.tensor.matmul(ss[:, t * 512:(t + 1) * 512], kT(h, c),
                                 qT(h, nb), start=True, stop=True)
            ptile = pt_pool.tile([128, 1024], BF16, name="pt", tag="pt")
            nc.scalar.activation(ptile[:, :], ss[:, :], AF.Exp, scale=SCALE)
            return ptile

        def av_piece(h, j, oo, ptile):
            for t in (0, 1):
                c = 2 * j + t
                nc.tensor.matmul(oo[:, :], vv[:, h, c, :],
                                 ptile[:, t * 512:(t + 1) * 512],
                                 start=(c == 0), stop=(c == NKC - 1))

        def norm(h, nb, oo):
            # recip of denominator row (partition 64), then broadcast to
            # partitions 0-63 via a one-hot stationary matmul on PE (much
            # shorter chain than a shift-DMA + gpsimd broadcast)
            nsl = slice(nb * QB, (nb + 1) * QB)
            rs = rs_pool.tile([65, QB], FP32, name="rs", tag="rs")
            nc.vector.reciprocal(rs[64:65, :], oo[64:65, :])
            r0 = rs_pool.tile([1, QB], FP32, name="r0", tag="r0")
            nc.sync.dma_start(out=r0[:, :], in_=rs[64:65, :])
            bcs = bc_pool.tile([64, QB], FP32, name="bc", tag="bc")
            nc.gpsimd.partition_broadcast(bcs[:, :], r0[:, :])
            if h == 0:
                nc.vector.tensor_mul(outT0[:, nsl], oo[0:64, :], bcs[:, :])
            elif h == 1:
                nc.vector.tensor_mul(outT12[0:64, nsl], oo[0:64, :],
                                     bcs[:, :])
            else:
                o2s = o2s_pool.tile([64, QB], FP32R, name="o2s", tag="o2s")
                nc.vector.tensor_mul(o2s[:, :], oo[0:64, :], bcs[:, :])
                nc.sync.dma_start(out=outT12[64:128, nsl], in_=o2s[:, :])

        def proj_block(nb):
            for l in range(4):
                ch = nb * 4 + l
                csl = slice(ch * 128, (ch + 1) * 128)
                for p in (0, 1):
                    osl = slice(p * 384, (p + 1) * 384)
                    ff = ps_f.tile([128, 384], FP32, name="ff", tag="ff")
                    nc.tensor.matmul(ff[:, :], outT0[:, csl], wp0[:, osl],
                                     start=True, stop=False)
                    nc.tensor.matmul(ff[:, :], outT12[:, csl], wp12[:, osl],
                                     start=False, stop=True)
                    fs = fs_pool.tile([128, 384], FP32, name="fs", tag="fs")
                    nc.vector.tensor_copy(fs[:, :], ff[:, :])
                    nc.sync.dma_start(out=out_d[csl, osl], in_=fs[:, :])

        # ---- stage A: streamed qkv projection + early head-a/block-0 ----
        with tc.tile_pool(name="psA", bufs=2, space="PSUM") as psA:
            nc.vector.memset(ones[:, :], 1.0)
            nc.vector.tensor_copy(
                vv[:, :, :, 64],
                ones[:, 0:48].rearrange("p (h c) -> p h c", h=NH)[:, :, :])
            warm = rs_pool.tile([65, 16], FP32, name="warm", tag="rs")
            nc.scalar.activation(warm[0:65, :], ones[0:65, 0:16], AF.Exp)

            oo_a0 = ps_o.tile([65, QB], FP32, name="oo", tag="oo")
            oo_c0 = ps_o.tile([65, QB], FP32, name="oo", tag="oo")
            pts_a0 = {}
            pts_b0 = {}
            pts_c0 = {}
            for Q in range(4):
                xq = xq_pool.tile([128, 6, QB], FP32R, name="xq", tag="xq")
                for cc in range(6):
                    if Q == 0:
                        nc.sync.dma_start(
                            out=wq_sb[:, cc, :],
                            in_=wqkvT_d[cc * 128:(cc + 1) * 128, :])
                    nc.sync.dma_start(
                        out=xq[:, cc, :],
                        in_=x_d[cc * 128:(cc + 1) * 128,
                                Q * QB:(Q + 1) * QB])
                # g0/g1 interleaved per cc so scores can start right after
                ps0 = psA.tile([128, QB], FP32, name="psqk", tag="psA")
                ps1 = psA.tile([128, QB], FP32, name="psqk", tag="psA")
                for cc in range(6):
                    nc.tensor.matmul(ps0[:, :], wq_sb[:, cc, 0:128],
                                     xq[:, cc, :],
                                     start=(cc == 0), stop=(cc == 5))
                    nc.tensor.matmul(ps1[:, :], wq_sb[:, cc, 128:256],
                                     xq[:, cc, :],
                                     start=(cc == 0), stop=(cc == 5))
                nc.scalar.activation(G[0][Q][:, :], ps0[:, :], AF.Copy)
                nc.scalar.activation(G[1][Q][:, :], ps1[:, :], AF.Copy)
                # heads a and b, block 0: scores for this quarter's chunks
                for j in (Q * 2, Q * 2 + 1):
                    pts_a0[j] = scores_piece(0, 0, j)
                    pts_b0[j] = scores_piece(1, 0, j)
                ps2 = psA.tile([128, QB], FP32, name="psqk", tag="psA")
                for cc in range(6):
                    nc.tensor.matmul(ps2[:, :], wq_sb[:, cc, 256:384],
                                     xq[:, cc, :],
                                     start=(cc == 0), stop=(cc == 5))
                nc.scalar.activation(G[2][Q][:, :], ps2[:, :], AF.Copy)
                nc.scalar.dma_start(out=algk2[Q][:, :],
                                    in_=G[2][Q][64:128, :])
                # head c scores need algk2 of this quarter
                for j in (Q * 2, Q * 2 + 1):
                    pts_c0[j] = scores_piece(2, 0, j)
                for l in range(4):
                    nb = Q * 4 + l
                    psv = psA.tile([128, QB], FP32, name="psv", tag="psA")
                    for cc in range(6):
                        nc.tensor.matmul(
                            psv[:, 0:256],
                            xq[:, cc, l * 128:(l + 1) * 128],
                            wq_sb[:, cc, 320:576],
                            start=(cc == 0), stop=(cc == 5))
                    nc.vector.tensor_copy(vv[:, :, nb, 0:64],
                                          psv[:, 64:256])
                # heads a and c avs lag one quarter (vv of Q-1 ready);
                # catch up fully at Q3
                avj = [Q * 2 - 2, Q * 2 - 1]
                if Q == 3:
                    avj += [6, 7]
                for j in avj:
                    if j >= 0:
                        av_piece(0, j, oo_a0, pts_a0.pop(j))
                        av_piece(2, j, oo_c0, pts_c0.pop(j))
                if Q == 0:
                    nc.sync.dma_start(out=wp0[:, :], in_=wpT_d[0:64, :])
                    nc.sync.dma_start(out=wp12[:, :], in_=wpT_d[64:192, :])

        ps_f = ctx.enter_context(tc.tile_pool(name="ps_f", bufs=2,
                                              space="PSUM"))
        norm(0, 0, oo_a0)
        norm(2, 0, oo_c0)

        def make_proj_piece(nb, l, p, eng="v"):
            def f():
                ch = nb * 4 + l
                csl = slice(ch * 128, (ch + 1) * 128)
                osl = slice(p * 384, (p + 1) * 384)
                ff = ps_f.tile([128, 384], FP32, name="ff", tag="ff")
                nc.tensor.matmul(ff[:, :], outT0[:, csl], wp0[:, osl],
                                 start=True, stop=False)
                nc.tensor.matmul(ff[:, :], outT12[:, csl], wp12[:, osl],
                                 start=False, stop=True)
                fs = fs_pool.tile([128, 384], FP32, name="fs", tag="fs")
                if eng == "v":
                    nc.vector.tensor_copy(fs[:, :], ff[:, :])
                else:
                    nc.scalar.activation(fs[:, :], ff[:, :], AF.Copy)
                nc.sync.dma_start(out=out_d[csl, osl], in_=fs[:, :])
            return f

        fillers = []

        # ---- steady state: element e emits its scores; avs of the
        # previous element (whose exps are done) run interleaved, so PE
        # never chases the ACT engine within an element.
        seq = [(2, 1), (0, 1), (1, 1),
               (2, 2), (0, 2), (1, 2),
               (2, 3), (0, 3)]
        last_of_block = {(1, 0): 0, (1, 1): 1, (1, 2): 2}
        prev, prev_pts = (1, 0), pts_b0
        for ei, (h, nb) in enumerate(seq):
            oo_prev = ps_o.tile([65, QB], FP32, name="oo", tag="oo")
            pts = {}
            for j in range(8):
                av_piece(prev[0], j, oo_prev, prev_pts.pop(j))
                pts[j] = scores_piece(h, nb, j)
                if fillers and j % 2 == 1:
                    fillers.pop(0)()
            norm(prev[0], prev[1], oo_prev)
            if prev in last_of_block:
                fillers += [make_proj_piece(last_of_block[prev], l, p)
                            for l in range(4) for p in (0, 1)]
            prev, prev_pts = (h, nb), pts

        # ---- tail: element (1,3) processed as two 256-wide halves so the
        # last exp covers only a quarter of the block; avs of (0,3) ride
        # along in half a; projection follows each half's norm.
        def scores_piece4(half, j4):
            ss = ps_s.tile([128, 1024], FP32, name="ss", tag="ss")
            qmov = qT(1, 3)[:, half * 256:(half + 1) * 256]
            for t in range(4):
                c = 4 * j4 + t
                nc.tensor.matmul(ss[:, t * 256:(t + 1) * 256], kT(1, c),
                                 qmov, start=True, stop=True)
            ptile = pt_pool.tile([128, 1024], BF16, name="pt", tag="pt")
            nc.scalar.activation(ptile[:, :], ss[:, :], AF.Exp, scale=SCALE)
            return ptile

        def av_piece4(j4, oo, ptile):
            for t in range(4):
                c = 4 * j4 + t
                nc.tensor.matmul(oo[:, :], vv[:, 1, c, :],
                                 ptile[:, t * 256:(t + 1) * 256],
                                 start=(c == 0), stop=(c == NKC - 1))

        def norm_half(half, oo):
            hs = slice(half * 256, (half + 1) * 256)
            rs = rs_pool.tile([65, 256], FP32, name="rs", tag="rs")
            nc.vector.reciprocal(rs[64:65, :], oo[64:65, :])
            r0 = rs_pool.tile([1, 256], FP32, name="r0", tag="r0")
            nc.scalar.dma_start(out=r0[:, :], in_=rs[64:65, :])
            bcs = bc_pool.tile([64, 256], FP32, name="bc", tag="bc")
            nc.gpsimd.partition_broadcast(bcs[:, :], r0[:, :])
            for l in range(2):
                lsl = slice(l * 128, (l + 1) * 128)
                nsl = slice(3 * QB + half * 256 + l * 128,
                            3 * QB + half * 256 + (l + 1) * 128)
                nc.vector.tensor_mul(outT12[0:64, nsl], oo[0:64, lsl],
                                     bcs[:, lsl])
                for p in (0, 1):
                    make_proj_piece(3, 2 * half + l, p,
                                    eng="s" if p else "v")()

        oo_03 = ps_o.tile([65, QB], FP32, name="oo", tag="oo")
        ooh = [None, None]
        ptsh = {}
        for half in (0, 1):
            ooh[half] = ps_o.tile([65, 256], FP32, name="oo", tag="oo")
            for j4 in range(4):
                if half == 0:
                    av_piece(0, 2 * j4, oo_03, prev_pts.pop(2 * j4))
                    av_piece(0, 2 * j4 + 1, oo_03, prev_pts.pop(2 * j4 + 1))
                ptsh[(half, j4)] = scores_piece4(half, j4)
                if j4 >= 1:
                    av_piece4(j4 - 1, ooh[half], ptsh.pop((half, j4 - 1)))
                if fillers:
                    fillers.pop(0)()
            if half == 0:
                norm(0, 3, oo_03)
            av_piece4(3, ooh[half], ptsh.pop((half, 3)))
            norm_half(half, ooh[half])

    nc.compile()
    return nc


def get_nc():
    if "nc" not in _cache:
        _cache["nc"] = _build()
    return _cache["nc"]


def make_in_maps(x, W_qkv, W_proj):
    x = np.asarray(x, dtype=np.float32)
    W_qkv = np.asarray(W_qkv, dtype=np.float32)
    W_proj = np.asarray(W_proj, dtype=np.float32)
    in_maps = []
    for core in range(NCORES):
        b, g = divmod(core, 4)
        r0 = 3 * g * D
        q = [W_qkv[r0 + h * D:r0 + (h + 1) * D] for h in range(NH)]
        k = [W_qkv[C + r0 + h * D:C + r0 + (h + 1) * D] for h in range(NH)]
        v = W_qkv[2 * C + r0:2 * C + r0 + NH * D]
        wqkvT = np.ascontiguousarray(
            np.concatenate([q[0], q[1], k[0], k[1], q[2], k[2], v], 0).T)
        wpT = np.ascontiguousarray(W_proj[:, r0:r0 + NH * D].T)
        in_maps.append({"x": np.ascontiguousarray(x[b].T),
                        "wqkvT": wqkvT, "wpT": wpT})
    return in_maps


def run(x, W_qkv, W_proj, trace=False):
    from concourse.bass_utils import run_bass_kernel_spmd
    nc = get_nc()
    in_maps = make_in_maps(x, W_qkv, W_proj)
    res = run_bass_kernel_spmd(nc, in_maps, list(range(NCORES)), trace=trace)
    out = np.zeros((B, N, C), dtype=np.float32)
    for core in range(NCORES):
        out[core // 4] += res.results[core]["out"]
    return out, res


def kernel(x, W_qkv, W_proj):
    out, _ = run(x, W_qkv, W_proj)
    return out


# revision 41
# speedup vs baseline: 1.0463x; 1.0463x over previous
"""Multi-head attention (B=2, N=2048, C=768, H=12) on 8 TRN2 NeuronCores.

Sharding: batch x head-group. core = b*4 + g handles batch b, heads
(a, b, c) = (3g, 3g+1, 3g+2). Each core computes qkv projection for its
heads, attention, and a partial output projection (row-sharded W_proj);
host sums the 4 partials per batch.

Device algorithm (per core), all matmuls fp32r:
  stage A (streamed by n-quarters of 512, x DMA'd per quarter):
    qk proj into [d, n] layout: 3 groups [q_a|q_b], [k_a|k_b], [q_c|k_c]
    (host-permuted W columns so each head's q and k share a partition
    base: a -> 0, b -> 64; k_c realigned to base 0 via one SB2SB DMA)
    v proj directly into [n, d] layout (stationary = x chunk, moving =
    W cols 320:576 = [k_c pad | v], 256 wide for full fp32r rate), with
    a ones column per nk chunk appended (softmax denominator trick)
    head-a / block-0 attention interleaved into stage A so the ACT
    engine (exp) starts early
  steady state per (head h, nq-block nb of 512):
    scores S^T chunks = kT' qT (PSUM, 2x512 per piece), exp via ACT
    (scale=1/8, no max subtraction: logits ~ N(0,1))
    prior block's projection emitted between scores and AV (fills PE
    while ACT works), then AV: out_aug = [v|1]^T P^T over 16 nk chunks
    normalize: reciprocal (DVE) -> partition_broadcast from partition 64
    (Pool) -> multiply (DVE); head c shifted to outT12[64:128] via DMA
  projection: per 128-row chunk, 2 pieces of 384 cols; ff (PSUM) ->
    fs (SBUF, Pool copy) -> DRAM
"""
import sys
sys.path.insert(0, "/opt/trn_rl_repo")
import numpy as np

HEADS = 12
C = 768
D = 64
N = 2048
B = 2
NH = 3            # heads per core
SCALE = D ** -0.5
NCORES = 8

_cache = {}


def _build():
    import concourse.bass as bass
    import concourse.mybir as mybir
    from concourse import bacc
    from concourse.tile import TileContext
    from contextlib import ExitStack

    FP32 = mybir.dt.float32
    FP32R = mybir.dt.float32r
    BF16 = mybir.dt.bfloat16
    AF = mybir.ActivationFunctionType

    nc = bacc.Bacc("TRN2", target_bir_lowering=False, debug=False,
                   num_devices=NCORES)
    x_d = nc.declare_dram_parameter("x", [C, N], FP32R, isOutput=False)
    wqkvT_d = nc.declare_dram_parameter("wqkvT", [C, 576], FP32R,
                                        isOutput=False)
    wpT_d = nc.declare_dram_parameter("wpT", [192, C], FP32R, isOutput=False)
    out_d = nc.declare_dram_parameter("out", [N, C], FP32, isOutput=True)

    QB = 512          # nq block size (== n quarter)
    NKC = 16          # nk chunks of 128

    with TileContext(nc) as tc, ExitStack() as ctx:
        persist = ctx.enter_context(tc.tile_pool(name="persist", bufs=1))
        ones = persist.tile([128, 64], FP32, name="ones")
        wq_sb = persist.tile([128, 6, 576], FP32R, name="wq_sb")
        # q/k group tiles per quarter: G0=[q_a|q_b] G1=[k_a|k_b] G2=[q_c|k_c]
        G = [[persist.tile([128, QB], FP32R, name=f"G{g}q{q}")
              for q in range(4)] for g in range(3)]
        algk2 = [persist.tile([64, QB], FP32R, name=f"algk2q{q}")
                 for q in range(4)]
        # v in nk-partition layout, 65th col = ones, all heads in one tile
        v_aug = persist.tile([128, NH * NKC * 65], BF16, name="v_aug")
        vv = v_aug.rearrange("p (h c w) -> p h c w", h=NH, w=65)
        outT0 = persist.tile([64, N], FP32R, name="outT0")
        outT12 = persist.tile([128, N], FP32R, name="outT12")
        wp0 = persist.tile([64, C], FP32R, name="wp0")
        wp12 = persist.tile([128, C], FP32R, name="wp12")

        xq_pool = ctx.enter_context(tc.tile_pool(name="xq", bufs=2))
        pt_pool = ctx.enter_context(tc.tile_pool(name="pt", bufs=24))
        rs_pool = ctx.enter_context(tc.tile_pool(name="rs", bufs=2))
        bc_pool = ctx.enter_context(tc.tile_pool(name="bc", bufs=2))
        fs_pool = ctx.enter_context(tc.tile_pool(name="fs", bufs=4))
        o2s_pool = ctx.enter_context(tc.tile_pool(name="o2s", bufs=2))
        ps_s = ctx.enter_context(tc.tile_pool(name="ps_s", bufs=2,
                                              space="PSUM"))
        ps_o = ctx.enter_context(tc.tile_pool(name="ps_o", bufs=2,
                                              space="PSUM"))

        def kT(h, c):          # stationary [64, 128], nk chunk c
            q, loc = divmod(c, 4)
            csl = slice(loc * 128, (loc + 1) * 128)
            if h == 0:
                return G[1][q][0:64, csl]
            if h == 1:
                return G[1][q][64:128, csl]
            return algk2[q][:, csl]

        def qT(h, nb):         # moving [64, 512], nq block nb
            if h == 0:
                return G[0][nb][0:64, :]
            if h == 1:
                return G[0][nb][64:128, :]
            return G[2][nb][0:64, :]

        def scores_piece(h, nb, j):
            ss = ps_s.tile([128, 1024], FP32, name="ss", tag="ss")
            for t in (0, 1):
                c = 2 * j + t
                nc.tensor.matmul(ss[:, t * 512:(t + 1) * 512], kT(h, c),
                                 qT(h, nb), start=True, stop=True)
            ptile = pt_pool.tile([128, 1024], BF16, name="pt", tag="pt")
            nc.scalar.activation(ptile[:, :], ss[:, :], AF.Exp, scale=SCALE)
            return ptile

        def av_piece(h, j, oo, ptile):
            for t in (0, 1):
                c = 2 * j + t
                nc.tensor.matmul(oo[:, :], vv[:, h, c, :],
                                 ptile[:, t * 512:(t + 1) * 512],
                                 start=(c == 0), stop=(c == NKC - 1))

        def norm(h, nb, oo):
            # recip of denominator row (partition 64), then broadcast to
            # partitions 0-63 via a one-hot stationary matmul on PE (much
            # shorter chain than a shift-DMA + gpsimd broadcast)
            nsl = slice(nb * QB, (nb + 1) * QB)
            rs = rs_pool.tile([65, QB], FP32, name="rs", tag="rs")
            nc.vector.reciprocal(rs[64:65, :], oo[64:65, :])
            r0 = rs_pool.tile([1, QB], FP32, name="r0", tag="r0")
            nc.sync.dma_start(out=r0[:, :], in_=rs[64:65, :])
            bcs = bc_pool.tile([64, QB], FP32, name="bc", tag="bc")
            nc.gpsimd.partition_broadcast(bcs[:, :], r0[:, :])
            if h == 0:
                nc.vector.tensor_mul(outT0[:, nsl], oo[0:64, :], bcs[:, :])
            elif h == 1:
                nc.vector.tensor_mul(outT12[0:64, nsl], oo[0:64, :],
                                     bcs[:, :])
            else:
                o2s = o2s_pool.tile([64, QB], FP32R, name="o2s", tag="o2s")
                nc.vector.tensor_mul(o2s[:, :], oo[0:64, :], bcs[:, :])
                nc.sync.dma_start(out=outT12[64:128, nsl], in_=o2s[:, :])

        def proj_block(nb):
            for l in range(4):
                ch = nb * 4 + l
                csl = slice(ch * 128, (ch + 1) * 128)
                for p in (0, 1):
                    osl = slice(p * 384, (p + 1) * 384)
                    ff = ps_f.tile([128, 384], FP32, name="ff", tag="ff")
                    nc.tensor.matmul(ff[:, :], outT0[:, csl], wp0[:, osl],
                                     start=True, stop=False)
                    nc.tensor.matmul(ff[:, :], outT12[:, csl], wp12[:, osl],
                                     start=False, stop=True)
                    fs = fs_pool.tile([128, 384], FP32, name="fs", tag="fs")
                    nc.vector.tensor_copy(fs[:, :], ff[:, :])
                    nc.sync.dma_start(out=out_d[csl, osl], in_=fs[:, :])

        # ---- stage A: streamed qkv projection + early head-a/block-0 ----
        with tc.tile_pool(name="psA", bufs=2, space="PSUM") as psA:
            nc.vector.memset(ones[:, :], 1.0)
            nc.vector.tensor_copy(
                vv[:, :, :, 64],
                ones[:, 0:48].rearrange("p (h c) -> p h c", h=NH)[:, :, :])
            warm = rs_pool.tile([65, 16], FP32, name="warm", tag="rs")
            nc.scalar.activation(warm[0:65, :], ones[0:65, 0:16], AF.Exp)

            oo_a0 = ps_o.tile([65, QB], FP32, name="oo", tag="oo")
            oo_c0 = ps_o.tile([65, QB], FP32, name="oo", tag="oo")
            pts_a0 = {}
            pts_b0 = {}
            pts_c0 = {}
            for Q in range(4):
                xq = xq_pool.tile([128, 6, QB], FP32R, name="xq", tag="xq")
                for cc in range(6):
                    if Q == 0:
                        nc.sync.dma_start(
                            out=wq_sb[:, cc, :],
                            in_=wqkvT_d[cc * 128:(cc + 1) * 128, :])
                    nc.sync.dma_start(
                        out=xq[:, cc, :],
                        in_=x_d[cc * 128:(cc + 1) * 128,
                                Q * QB:(Q + 1) * QB])
                # g0/g1 interleaved per cc so scores can start right after
                ps0 = psA.tile([128, QB], FP32, name="psqk", tag="psA")
                ps1 = psA.tile([128, QB], FP32, name="psqk", tag="psA")
                for cc in range(6):
                    nc.tensor.matmul(ps0[:, :], wq_sb[:, cc, 0:128],
                                     xq[:, cc, :],
                                     start=(cc == 0), stop=(cc == 5))
                    nc.tensor.matmul(ps1[:, :], wq_sb[:, cc, 128:256],
                                     xq[:, cc, :],
                                     start=(cc == 0), stop=(cc == 5))
                nc.vector.tensor_copy(G[0][Q][:, :], ps0[:, :])
                nc.vector.tensor_copy(G[1][Q][:, :], ps1[:, :])
                # heads a and b, block 0: scores for this quarter's chunks
                for j in (Q * 2, Q * 2 + 1):
                    pts_a0[j] = scores_piece(0, 0, j)
                    pts_b0[j] = scores_piece(1, 0, j)
                ps2 = psA.tile([128, QB], FP32, name="psqk", tag="psA")
                for cc in range(6):
                    nc.tensor.matmul(ps2[:, :], wq_sb[:, cc, 256:384],
                                     xq[:, cc, :],
                                     start=(cc == 0), stop=(cc == 5))
                nc.vector.tensor_copy(G[2][Q][:, :], ps2[:, :])
                nc.scalar.dma_start(out=algk2[Q][:, :],
                                    in_=G[2][Q][64:128, :])
                # head c scores need algk2 of this quarter
                for j in (Q * 2, Q * 2 + 1):
                    pts_c0[j] = scores_piece(2, 0, j)
                for l in range(4):
                    nb = Q * 4 + l
                    psv = psA.tile([128, QB], FP32, name="psv", tag="psA")
                    for cc in range(6):
                        nc.tensor.matmul(
                            psv[:, 0:256],
                            xq[:, cc, l * 128:(l + 1) * 128],
                            wq_sb[:, cc, 320:576],
                            start=(cc == 0), stop=(cc == 5))
                    nc.vector.tensor_copy(vv[:, :, nb, 0:64],
                                          psv[:, 64:256])
                # heads a and c avs lag one quarter (vv of Q-1 ready);
                # catch up fully at Q3
                avj = [Q * 2 - 2, Q * 2 - 1]
                if Q == 3:
                    avj += [6, 7]
                for j in avj:
                    if j >= 0:
                        av_piece(0, j, oo_a0, pts_a0.pop(j))
                        av_piece(2, j, oo_c0, pts_c0.pop(j))
                if Q == 0:
                    nc.sync.dma_start(out=wp0[:, :], in_=wpT_d[0:64, :])
                    nc.sync.dma_start(out=wp12[:, :], in_=wpT_d[64:192, :])

        ps_f = ctx.enter_context(tc.tile_pool(name="ps_f", bufs=2,
                                              space="PSUM"))
        norm(0, 0, oo_a0)
        norm(2, 0, oo_c0)

        def make_proj_piece(nb, l, p, eng="v"):
            def f():
                ch = nb * 4 + l
                csl = slice(ch * 128, (ch + 1) * 128)
                osl = slice(p * 384, (p + 1) * 384)
                ff = ps_f.tile([128, 384], FP32, name="ff", tag="ff")
                nc.tensor.matmul(ff[:, :], outT0[:, csl], wp0[:, osl],
                                 start=True, stop=False)
                nc.tensor.matmul(ff[:, :], outT12[:, csl], wp12[:, osl],
                                 start=False, stop=True)
                fs = fs_pool.tile([128, 384], FP32, name="fs", tag="fs")
                if eng == "v":
                    nc.vector.tensor_copy(fs[:, :], ff[:, :])
                else:
                    nc.scalar.activation(fs[:, :], ff[:, :], AF.Copy)
                nc.sync.dma_start(out=out_d[csl, osl], in_=fs[:, :])
            return f

        fillers = []

        # ---- steady state: element e emits its scores; avs of the
        # previous element (whose exps are done) run interleaved, so PE
        # never chases the ACT engine within an element.
        seq = [(2, 1), (0, 1), (1, 1),
               (2, 2), (0, 2), (1, 2),
               (2, 3), (0, 3)]
        last_of_block = {(1, 0): 0, (1, 1): 1, (1, 2): 2}
        prev, prev_pts = (1, 0), pts_b0
        for ei, (h, nb) in enumerate(seq):
            oo_prev = ps_o.tile([65, QB], FP32, name="oo", tag="oo")
            pts = {}
            for j in range(8):
                pts[j] = scores_piece(h, nb, j)
                av_piece(prev[0], j, oo_prev, prev_pts.pop(j))
                if fillers and j % 2 == 1:
                    fillers.pop(0)()
            norm(prev[0], prev[1], oo_prev)
            if prev in last_of_block:
                fillers += [make_proj_piece(last_of_block[prev], l, p)
                            for l in range(4) for p in (0, 1)]
            prev, prev_pts = (h, nb), pts

        # ---- tail: element (1,3) processed as two 256-wide halves so the
        # last exp covers only a quarter of the block; avs of (0,3) ride
        # along in half a; projection follows each half's norm.
        def scores_piece4(half, j4):
            ss = ps_s.tile([128, 1024], FP32, name="ss", tag="ss")
            qmov = qT(1, 3)[:, half * 256:(half + 1) * 256]
            for t in range(4):
                c = 4 * j4 + t
                nc.tensor.matmul(ss[:, t * 256:(t + 1) * 256], kT(1, c),
                                 qmov, start=True, stop=True)
            ptile = pt_pool.tile([128, 1024], BF16, name="pt", tag="pt")
            nc.scalar.activation(ptile[:, :], ss[:, :], AF.Exp, scale=SCALE)
            return ptile

        def av_piece4(j4, oo, ptile):
            for t in range(4):
                c = 4 * j4 + t
                nc.tensor.matmul(oo[:, :], vv[:, 1, c, :],
                                 ptile[:, t * 256:(t + 1) * 256],
                                 start=(c == 0), stop=(c == NKC - 1))

        def norm_half(half, oo):
            hs = slice(half * 256, (half + 1) * 256)
            rs = rs_pool.tile([65, 256], FP32, name="rs", tag="rs")
            nc.vector.reciprocal(rs[64:65, :], oo[64:65, :])
            r0 = rs_pool.tile([1, 256], FP32, name="r0", tag="r0")
            nc.scalar.dma_start(out=r0[:, :], in_=rs[64:65, :])
            bcs = bc_pool.tile([64, 256], FP32, name="bc", tag="bc")
            nc.gpsimd.partition_broadcast(bcs[:, :], r0[:, :])
            for l in range(2):
                lsl = slice(l * 128, (l + 1) * 128)
                nsl = slice(3 * QB + half * 256 + l * 128,
                            3 * QB + half * 256 + (l + 1) * 128)
                nc.vector.tensor_mul(outT12[0:64, nsl], oo[0:64, lsl],
                                     bcs[:, lsl])
                for p in (0, 1):
                    make_proj_piece(3, 2 * half + l, p,
                                    eng="s" if p else "v")()

        oo_03 = ps_o.tile([65, QB], FP32, name="oo", tag="oo")
        ooh = [None, None]
        ptsh = {}
        for half in (0, 1):
            ooh[half] = ps_o.tile([65, 256], FP32, name="oo", tag="oo")
            for j4 in range(4):
                if half == 0:
                    av_piece(0, 2 * j4, oo_03, prev_pts.pop(2 * j4))
                    av_piece(0, 2 * j4 + 1, oo_03, prev_pts.pop(2 * j4 + 1))
                ptsh[(half, j4)] = scores_piece4(half, j4)
                if j4 >= 1:
                    av_piece4(j4 - 1, ooh[half], ptsh.pop((half, j4 - 1)))
                if fillers:
                    fillers.pop(0)()
            if half == 0:
                norm(0, 3, oo_03)
            av_piece4(3, ooh[half], ptsh.pop((half, 3)))
            norm_half(half, ooh[half])

    nc.compile()
    return nc


def get_nc():
    if "nc" not in _cache:
        _cache["nc"] = _build()
    return _cache["nc"]


def make_in_maps(x, W_qkv, W_proj):
    x = np.asarray(x, dtype=np.float32)
    W_qkv = np.asarray(W_qkv, dtype=np.float32)
    W_proj = np.asarray(W_proj, dtype=np.float32)
    in_maps = []
    for core in range(NCORES):
        b, g = divmod(core, 4)
        r0 = 3 * g * D
        q = [W_qkv[r0 + h * D:r0 + (h + 1) * D] for h in range(NH)]
        k = [W_qkv[C + r0 + h * D:C + r0 + (h + 1) * D] for h in range(NH)]
        v = W_qkv[2 * C + r0:2 * C + r0 + NH * D]
        wqkvT = np.ascontiguousarray(
            np.concatenate([q[0], q[1], k[0], k[1], q[2], k[2], v], 0).T)
        wpT = np.ascontiguousarray(W_proj[:, r0:r0 + NH * D].T)
        in_maps.append({"x": np.ascontiguousarray(x[b].T),
                        "wqkvT": wqkvT, "wpT": wpT})
    return in_maps


def run(x, W_qkv, W_proj, trace=False):
    from concourse.bass_utils import run_bass_kernel_spmd
    nc = get_nc()
    in_maps = make_in_maps(x, W_qkv, W_proj)
    res = run_bass_kernel_spmd(nc, in_maps, list(range(NCORES)), trace=trace)
    out = np.zeros((B, N, C), dtype=np.float32)
    for core in range(NCORES):
        out[core // 4] += res.results[core]["out"]
    return out, res


def kernel(x, W_qkv, W_proj):
    out, _ = run(x, W_qkv, W_proj)
    return out
